# revision 21
# baseline (speedup 1.0000x reference)
"""Trainium2 Bass kernel for BinaryLinear: y = x @ sign(weight).T

Full shapes: x [32, 4096, 1024] f32, weight [1024, 1024] f32 -> y [32, 4096, 1024] f32.
Sharding: data-parallel over tokens across 8 NeuronCores (16384 tokens each).

All data reshaping is done on host so the device kernel is a pure matmul stream:
  - x is sharded, transposed to [feature, token], cast f16 (and the first
    256*FP8_CHUNKS features additionally packed as fp8e4m3 pairs for
    DoubleRow double-pumped matmuls).
  - weight is sign()ed, transposed and packed on host (exact in f16/fp8).
  - y comes back as yT [1024, 16384] f16 per core and is untransposed on host.

Device kernel per core (weight-stationary, PE-bound):
  W resides in SBUF; for each 512-token tile: load xT tile, run
  8 o_chunks x (FP8_CHUNKS DoubleRow + remaining bf16) accumulating matmuls
  into a PSUM bank [128 o, 512 t], copy to SBUF f16 (alternating
  vector/scalar), DMA out.
"""

from contextlib import ExitStack

import numpy as np
import ml_dtypes

import concourse.bass as bass
import concourse.mybir as mybir
import concourse.tile as tile
from concourse import bacc
from concourse.bass import ts
from concourse.bass_utils import run_bass_kernel_spmd

P = 128
N_CORES = 8
F32 = mybir.dt.float32
F16 = mybir.dt.float16
F8 = mybir.dt.float8e4

FULL_B, FULL_S, D_IN = 32, 4096, 1024
D_OUT = 1024
TOKENS_PER_CORE = FULL_B * FULL_S // N_CORES  # 16384

TT = 512                     # tokens per tile (one PSUM bank of f32)
FP8_CHUNKS = 2               # 256-wide contraction superchunks done in fp8 DoubleRow
NP_F8 = ml_dtypes.float8_e4m3
NP_F16 = np.float16


def build_nc(tokens=TOKENS_PER_CORE, d_in=D_IN, d_out=D_OUT, fp8_chunks=FP8_CHUNKS):
    """Per-core program: yT[o, t] = sum_i sign(w)[o, i] * x[t, i]."""
    d8 = 256 * fp8_chunks            # features carried by fp8 DoubleRow
    d16 = d_in - d8                  # features carried by f16
    k16 = d16 // P                   # f16 contraction chunks
    o_ch = d_out // P
    n_t = tokens // TT

    nc = bacc.Bacc("TRN2")
    if d16:
        xT = nc.dram_tensor("xT", [d16, tokens], F16, kind="ExternalInput")
        wT = nc.dram_tensor("wT", [d16, d_out], F16, kind="ExternalInput")
    if d8:
        # x8 rows: [c*128 + i]; per row the two pair features are byte-adjacent
        # ([t, pair] order) so DoubleRow streams contiguous bytes.
        x8 = nc.dram_tensor("x8", [d8 // 2, 2 * tokens], F8, kind="ExternalInput")
        w8 = nc.dram_tensor("w8", [d8, d_out], F8, kind="ExternalInput")
    y = nc.dram_tensor("y", [d_out, tokens], F16, kind="ExternalOutput")

    PF = min(4, n_t)  # x prefetch depth (tiles)

    with tile.TileContext(nc) as tc, ExitStack() as ctx:
        wpool = ctx.enter_context(tc.tile_pool(name="w", bufs=1))
        xpool = ctx.enter_context(tc.tile_pool(name="xin", bufs=PF + 1))
        pspool = ctx.enter_context(tc.tile_pool(name="ps", bufs=6, space="PSUM"))
        wpspool = ctx.enter_context(tc.tile_pool(name="wpsp", bufs=1, space="PSUM"))
        opool = ctx.enter_context(tc.tile_pool(name="out", bufs=6))

        if d16:
            xT_g = xT.rearrange("(kc p) (g t) -> g p kc t", p=P, t=TT)
            wT_r = wT.rearrange("(kc p) o -> p kc o", p=P)
        if d8:
            x8_g = x8.rearrange("(c p) (g t pr) -> g p c t pr", p=P, pr=2, t=TT)
            w8_r = w8.rearrange("(c pr p) o -> p c pr o", p=P, pr=2)
        y_g = y.rearrange("(oc p) (g t) -> oc g p t", p=P, t=TT)

        xts = {}

        def load_x(g, fine=False):
            # returns ([f16 chunk APs], [fp8 chunk APs]); fine=True uses one
            # tile per chunk so dependencies (tile-granular) are minimal for
            # the pipeline prologue
            aps16, aps8 = [], []
            if d16:
                if fine:
                    for kc in range(k16):
                        t = xpool.tile([P, TT], F16, name="x16f", tag=f"x16f{kc}")
                        eng = nc.sync if kc % 2 == 0 else nc.gpsimd
                        eng.dma_start(t, xT_g[g, :, kc, :])
                        aps16.append(t)
                else:
                    t16 = xpool.tile([P, k16, TT], F16, name="x16t", tag="x16t")
                    for h in range(0, k16, 2):
                        hw_ = min(2, k16 - h)
                        nc.sync.dma_start(
                            t16[:, h : h + hw_, :], xT_g[g, :, h : h + hw_, :]
                        )
                    aps16 = [t16[:, kc, :] for kc in range(k16)]
            if d8:
                if fine:
                    for c in range(fp8_chunks):
                        t = xpool.tile([P, TT, 2], F8, name="x8f", tag=f"x8f{c}")
                        nc.sync.dma_start(t, x8_g[g, :, c, :, :])
                        aps8.append(t)
                else:
                    t8 = xpool.tile([P, fp8_chunks, TT, 2], F8, name="x8t", tag="x8t")
                    for c in range(fp8_chunks):
                        nc.sync.dma_start(t8[:, c, :, :], x8_g[g, :, c, :, :])
                    aps8 = [t8[:, c, :, :] for c in range(fp8_chunks)]
            xts[g] = (aps16, aps8)

        # PE warm-up: a burst of scratch matmuls during the DMA prologue trips
        # the HAM clock-gate to full rate before the real matmuls arrive
        warm = wpool.tile([P, TT], F16, name="warm")
        nc.vector.memset(warm, 0.0)
        wps = wpspool.tile([P, TT], F32, name="wps")
        for _ in range(10):
            nc.tensor.matmul(wps, warm[:, :P], warm, start=True, stop=True)

        # one-time weight loads into SBUF (replicated weights are small);
        # one tile per chunk keeps the first matmul's wait minimal
        W16s, W8s = [], []
        if d16:
            for kc in range(k16):
                t = wpool.tile([P, d_out], F16, name=f"W16_{kc}")
                nc.scalar.dma_start(t, wT_r[:, kc, :])
                W16s.append(t)
        if d8:
            for c in range(fp8_chunks):
                t = wpool.tile([P, 2, d_out], F8, name=f"W8_{c}")
                nc.scalar.dma_start(t, w8_r[:, c, :, :])
                W8s.append(t)

        for g in range(PF):
            load_x(g, fine=(g == 0))

        n_mm = fp8_chunks + k16
        for g in range(n_t):
            if g + PF < n_t:
                load_x(g + PF)
            aps16, aps8 = xts.pop(g)
            for oc in range(o_ch):
                ps = pspool.tile([P, TT], F32, name="ps")
                mm = 0
                for kc in range(k16):
                    nc.tensor.matmul(
                        ps,
                        W16s[kc][:, ts(oc, P)],
                        aps16[kc],
                        start=(mm == 0),
                        stop=(mm == n_mm - 1),
                    )
                    mm += 1
                for c in range(fp8_chunks):
                    nc.tensor.matmul(
                        ps,
                        W8s[c][:, :, ts(oc, P)],
                        aps8[c].rearrange("p t pr -> p pr t"),
                        start=(mm == 0),
                        stop=(mm == n_mm - 1),
                        perf_mode=mybir.MatmulPerfMode.DoubleRow,
                    )
                    mm += 1
                out = opool.tile([P, TT], F16, name="out")
                if g == n_t - 1 and oc == o_ch - 1:
                    # final group: split the store across two queues to cut the tail
                    H = TT // 2
                    nc.vector.tensor_copy(out[:, :H], ps[:, :H])
                    nc.sync.dma_start(y_g[oc, g][:, :H], out[:, :H])
                    nc.vector.tensor_copy(out[:, H:], ps[:, H:])
                    nc.scalar.dma_start(y_g[oc, g][:, H:], out[:, H:])
                elif oc % 2 == 0:
                    nc.vector.tensor_copy(out, ps)
                    nc.gpsimd.dma_start(y_g[oc, g], out)
                else:
                    nc.scalar.copy(out, ps)
                    nc.sync.dma_start(y_g[oc, g], out)
    nc.compile()
    return nc


_NC_CACHE = {}


def _get_nc():
    key = (TOKENS_PER_CORE, D_IN, D_OUT, FP8_CHUNKS)
    if key not in _NC_CACHE:
        _NC_CACHE[key] = build_nc()
    return _NC_CACHE[key]


def _prep_inputs(x, weight):
    """Host-side shard + transpose + cast. Returns per-core input maps."""
    d8 = 256 * FP8_CHUNKS
    ws = np.sign(weight)  # [o, i]
    wsT = np.ascontiguousarray(ws.T)  # [i, o]
    base = {}
    if d8 < D_IN:
        base["wT"] = wsT[d8:].astype(NP_F16)
    if d8:
        base["w8"] = wsT[:d8].astype(NP_F8)

    x_flat = x.reshape(N_CORES, TOKENS_PER_CORE, D_IN)
    in_maps = []
    for c in range(N_CORES):
        xc = x_flat[c]  # [t, i]
        m = dict(base)
        if d8 < D_IN:
            m["xT"] = np.ascontiguousarray(xc[:, d8:].T, dtype=NP_F16)
        if d8:
            # pack [c*128+i, 2*t + pair]: pair features (256c+128*pr+i) byte-adjacent
            a = xc[:, :d8].astype(NP_F8)  # [t, d8]
            a = a.reshape(TOKENS_PER_CORE, FP8_CHUNKS, 2, P)  # [t, c, pr, i]
            a = a.transpose(1, 3, 0, 2)  # [c, i, t, pr]
            m["x8"] = np.ascontiguousarray(a.reshape(d8 // 2, 2 * TOKENS_PER_CORE))
        in_maps.append(m)
    return in_maps


def run(x, weight, trace=False, **kwargs):
    """Shard, execute on 8 cores, gather. Returns (y_full, BassKernelResults)."""
    x = np.ascontiguousarray(x, dtype=np.float32)
    weight = np.ascontiguousarray(weight, dtype=np.float32)
    assert x.shape == (FULL_B, FULL_S, D_IN), x.shape
    assert weight.shape == (D_OUT, D_IN), weight.shape

    in_maps = _prep_inputs(x, weight)
    nc = _get_nc()
    res = run_bass_kernel_spmd(
        nc, in_maps, core_ids=list(range(N_CORES)), trace=trace, **kwargs
    )
    y = np.empty((N_CORES, TOKENS_PER_CORE, D_OUT), dtype=np.float32)
    for c in range(N_CORES):
        y[c] = res.results[c]["y"].T.astype(np.float32)
    return y.reshape(FULL_B, FULL_S, D_OUT), res


def kernel(x, weight):
    try:
        y, _ = run(x, weight)
    except Exception:
        # A freshly-loaded NEFF occasionally faults on its first execution
        # (device-side NRT_EXEC_UNIT_UNRECOVERABLE); one retry has always
        # recovered in testing.
        y, _ = run(x, weight)
    return y


# revision 22
# speedup vs baseline: 1.0076x; 1.0076x over previous
"""Trainium2 Bass kernel for BinaryLinear: y = x @ sign(weight).T

Full shapes: x [32, 4096, 1024] f32, weight [1024, 1024] f32 -> y [32, 4096, 1024] f32.
Sharding: data-parallel over tokens across 8 NeuronCores (16384 tokens each).

All data reshaping is done on host so the device kernel is a pure matmul stream:
  - x is sharded, transposed to [feature, token], cast f16 (and the first
    256*FP8_CHUNKS features additionally packed as fp8e4m3 pairs for
    DoubleRow double-pumped matmuls).
  - weight is sign()ed, transposed and packed on host (exact in f16/fp8).
  - y comes back as yT [1024, 16384] f16 per core and is untransposed on host.

Device kernel per core (weight-stationary, PE-bound):
  W resides in SBUF; for each 512-token tile: load xT tile, run
  8 o_chunks x (FP8_CHUNKS DoubleRow + remaining bf16) accumulating matmuls
  into a PSUM bank [128 o, 512 t], copy to SBUF f16 (alternating
  vector/scalar), DMA out.
"""

from contextlib import ExitStack

import numpy as np
import ml_dtypes

import concourse.bass as bass
import concourse.mybir as mybir
import concourse.tile as tile
from concourse import bacc
from concourse.bass import ts
from concourse.bass_utils import run_bass_kernel_spmd

P = 128
N_CORES = 8
F32 = mybir.dt.float32
F16 = mybir.dt.float16
F8 = mybir.dt.float8e4

FULL_B, FULL_S, D_IN = 32, 4096, 1024
D_OUT = 1024
TOKENS_PER_CORE = FULL_B * FULL_S // N_CORES  # 16384

TT = 512                     # tokens per tile (one PSUM bank of f32)
FP8_CHUNKS = 2               # 256-wide contraction superchunks done in fp8 DoubleRow
NP_F8 = ml_dtypes.float8_e4m3
NP_F16 = np.float16


def build_nc(tokens=TOKENS_PER_CORE, d_in=D_IN, d_out=D_OUT, fp8_chunks=FP8_CHUNKS):
    """Per-core program: yT[o, t] = sum_i sign(w)[o, i] * x[t, i]."""
    d8 = 256 * fp8_chunks            # features carried by fp8 DoubleRow
    d16 = d_in - d8                  # features carried by f16
    k16 = d16 // P                   # f16 contraction chunks
    o_ch = d_out // P
    n_t = tokens // TT

    nc = bacc.Bacc("TRN2")
    if d16:
        xT = nc.dram_tensor("xT", [d16, tokens], F16, kind="ExternalInput")
        wT = nc.dram_tensor("wT", [d16, d_out], F16, kind="ExternalInput")
    if d8:
        # x8 rows: [c*128 + i]; per row the two pair features are byte-adjacent
        # ([t, pair] order) so DoubleRow streams contiguous bytes.
        x8 = nc.dram_tensor("x8", [d8 // 2, 2 * tokens], F8, kind="ExternalInput")
        w8 = nc.dram_tensor("w8", [d8, d_out], F8, kind="ExternalInput")
    y = nc.dram_tensor("y", [d_out, tokens], F16, kind="ExternalOutput")

    PF = min(4, n_t)  # x prefetch depth (tiles)

    with tile.TileContext(nc) as tc, ExitStack() as ctx:
        wpool = ctx.enter_context(tc.tile_pool(name="w", bufs=1))
        xpool = ctx.enter_context(tc.tile_pool(name="xin", bufs=PF + 1))
        pspool = ctx.enter_context(tc.tile_pool(name="ps", bufs=6, space="PSUM"))
        opool = ctx.enter_context(tc.tile_pool(name="out", bufs=6))

        if d16:
            xT_g = xT.rearrange("(kc p) (g t) -> g p kc t", p=P, t=TT)
            wT_r = wT.rearrange("(kc p) o -> p kc o", p=P)
        if d8:
            x8_g = x8.rearrange("(c p) (g t pr) -> g p c t pr", p=P, pr=2, t=TT)
            w8_r = w8.rearrange("(c pr p) o -> p c pr o", p=P, pr=2)
        y_g = y.rearrange("(oc p) (g t) -> oc g p t", p=P, t=TT)

        xts = {}

        def load_x(g, fine=False):
            # returns ([f16 chunk APs], [fp8 chunk APs]); fine=True uses one
            # tile per chunk so dependencies (tile-granular) are minimal for
            # the pipeline prologue
            aps16, aps8 = [], []
            if d16:
                if fine:
                    for kc in range(k16):
                        t = xpool.tile([P, TT], F16, name="x16f", tag=f"x16f{kc}")
                        eng = nc.sync if kc % 2 == 0 else nc.gpsimd
                        eng.dma_start(t, xT_g[g, :, kc, :])
                        aps16.append(t)
                else:
                    t16 = xpool.tile([P, k16, TT], F16, name="x16t", tag="x16t")
                    for h in range(0, k16, 2):
                        hw_ = min(2, k16 - h)
                        nc.sync.dma_start(
                            t16[:, h : h + hw_, :], xT_g[g, :, h : h + hw_, :]
                        )
                    aps16 = [t16[:, kc, :] for kc in range(k16)]
            if d8:
                if fine:
                    for c in range(fp8_chunks):
                        t = xpool.tile([P, TT, 2], F8, name="x8f", tag=f"x8f{c}")
                        nc.sync.dma_start(t, x8_g[g, :, c, :, :])
                        aps8.append(t)
                else:
                    t8 = xpool.tile([P, fp8_chunks, TT, 2], F8, name="x8t", tag="x8t")
                    for c in range(fp8_chunks):
                        nc.sync.dma_start(t8[:, c, :, :], x8_g[g, :, c, :, :])
                    aps8 = [t8[:, c, :, :] for c in range(fp8_chunks)]
            xts[g] = (aps16, aps8)

        # one-time weight loads into SBUF (replicated weights are small);
        # one tile per chunk keeps the first matmul's wait minimal
        W16s, W8s = [], []
        if d16:
            for kc in range(k16):
                t = wpool.tile([P, d_out], F16, name=f"W16_{kc}")
                nc.scalar.dma_start(t, wT_r[:, kc, :])
                W16s.append(t)
        if d8:
            for c in range(fp8_chunks):
                t = wpool.tile([P, 2, d_out], F8, name=f"W8_{c}")
                nc.scalar.dma_start(t, w8_r[:, c, :, :])
                W8s.append(t)

        for g in range(PF):
            load_x(g, fine=(g == 0))

        n_mm = fp8_chunks + k16
        for g in range(n_t):
            if g + PF < n_t:
                load_x(g + PF)
            aps16, aps8 = xts.pop(g)
            for oc in range(o_ch):
                ps = pspool.tile([P, TT], F32, name="ps")
                mm = 0
                for kc in range(k16):
                    nc.tensor.matmul(
                        ps,
                        W16s[kc][:, ts(oc, P)],
                        aps16[kc],
                        start=(mm == 0),
                        stop=(mm == n_mm - 1),
                    )
                    mm += 1
                for c in range(fp8_chunks):
                    nc.tensor.matmul(
                        ps,
                        W8s[c][:, :, ts(oc, P)],
                        aps8[c].rearrange("p t pr -> p pr t"),
                        start=(mm == 0),
                        stop=(mm == n_mm - 1),
                        perf_mode=mybir.MatmulPerfMode.DoubleRow,
                    )
                    mm += 1
                out = opool.tile([P, TT], F16, name="out")
                if g == n_t - 1 and oc == o_ch - 1:
                    # final group: split the store across two queues to cut the tail
                    H = TT // 2
                    nc.vector.tensor_copy(out[:, :H], ps[:, :H])
                    nc.sync.dma_start(y_g[oc, g][:, :H], out[:, :H])
                    nc.vector.tensor_copy(out[:, H:], ps[:, H:])
                    nc.scalar.dma_start(y_g[oc, g][:, H:], out[:, H:])
                elif oc % 2 == 0:
                    nc.vector.tensor_copy(out, ps)
                    nc.gpsimd.dma_start(y_g[oc, g], out)
                else:
                    nc.scalar.copy(out, ps)
                    nc.sync.dma_start(y_g[oc, g], out)
    nc.compile()
    return nc


_NC_CACHE = {}


def _get_nc():
    key = (TOKENS_PER_CORE, D_IN, D_OUT, FP8_CHUNKS)
    if key not in _NC_CACHE:
        _NC_CACHE[key] = build_nc()
    return _NC_CACHE[key]


def _prep_inputs(x, weight):
    """Host-side shard + transpose + cast. Returns per-core input maps."""
    d8 = 256 * FP8_CHUNKS
    ws = np.sign(weight)  # [o, i]
    wsT = np.ascontiguousarray(ws.T)  # [i, o]
    base = {}
    if d8 < D_IN:
        base["wT"] = wsT[d8:].astype(NP_F16)
    if d8:
        base["w8"] = wsT[:d8].astype(NP_F8)

    x_flat = x.reshape(N_CORES, TOKENS_PER_CORE, D_IN)
    in_maps = []
    for c in range(N_CORES):
        xc = x_flat[c]  # [t, i]
        m = dict(base)
        if d8 < D_IN:
            m["xT"] = np.ascontiguousarray(xc[:, d8:].T, dtype=NP_F16)
        if d8:
            # pack [c*128+i, 2*t + pair]: pair features (256c+128*pr+i) byte-adjacent
            a = xc[:, :d8].astype(NP_F8)  # [t, d8]
            a = a.reshape(TOKENS_PER_CORE, FP8_CHUNKS, 2, P)  # [t, c, pr, i]
            a = a.transpose(1, 3, 0, 2)  # [c, i, t, pr]
            m["x8"] = np.ascontiguousarray(a.reshape(d8 // 2, 2 * TOKENS_PER_CORE))
        in_maps.append(m)
    return in_maps


def run(x, weight, trace=False, **kwargs):
    """Shard, execute on 8 cores, gather. Returns (y_full, BassKernelResults)."""
    x = np.ascontiguousarray(x, dtype=np.float32)
    weight = np.ascontiguousarray(weight, dtype=np.float32)
    assert x.shape == (FULL_B, FULL_S, D_IN), x.shape
    assert weight.shape == (D_OUT, D_IN), weight.shape

    in_maps = _prep_inputs(x, weight)
    nc = _get_nc()
    res = run_bass_kernel_spmd(
        nc, in_maps, core_ids=list(range(N_CORES)), trace=trace, **kwargs
    )
    y = np.empty((N_CORES, TOKENS_PER_CORE, D_OUT), dtype=np.float32)
    for c in range(N_CORES):
        y[c] = res.results[c]["y"].T.astype(np.float32)
    return y.reshape(FULL_B, FULL_S, D_OUT), res


def kernel(x, weight):
    try:
        y, _ = run(x, weight)
    except Exception:
        # A freshly-loaded NEFF occasionally faults on its first execution
        # (device-side NRT_EXEC_UNIT_UNRECOVERABLE); one retry has always
        # recovered in testing.
        y, _ = run(x, weight)
    return y
